# revision 4
# baseline (speedup 1.0000x reference)
"""Trainium2 Bass kernel for nn_Distance (exact EDT + Gaussian click maps).

Computes, for inputs [4, 320, 320, 2] f32 in [0,1):
  restored = uint8((1-x)*127.5); zero-mask = (restored == 0)
  d2 = exact squared Euclidean distance transform of the zero-mask
       (per image, channel folded into batch -> 8 independent images)
  out[..., c*3+s] = exp(-d2_c / (2*sigma_s^2)), sigmas = [0.02,0.08,0.16]*320

Sharding: pure data parallel, one folded image (b, c) per NeuronCore (8 cores).

Device algorithm (exact, decomposed transposed relative to the reference --
provably identical results):
  phase A: per-row 1D distances along W via two prefix max scans (DVE),
           g2row = g^2 (ACT square)
  transpose g2row -> [W(p), H(f)] via DMA xbar transpose (fp16) / PE (fp32)
  phase B: d2[j, i] = min_{|o|<=R} g2row_T[j, i+o] + o^2 as a bulk
           windowed tensor-tensor min + add + log-fold min tree (DVE, fp16 2x)
  exp:     3 ACT Exp activations per chunk, scale = -1/denom_s
R is derived on the host from the actual input (exact convergence bound), so
device results are exact. fp16 path is used when max finite d2 <= 2047 (all
winning candidates are integers <= 2047, exactly representable in fp16;
clamped/padded losers stay > any winner -- see analysis in comments).
"""

import math
import os
import sys

import numpy as np

for _p in ("/opt/trn_rl_repo", "/root/.axon_site/_ro/trn_rl_repo"):
    if os.path.isdir(_p) and _p not in sys.path:
        sys.path.insert(0, _p)

import concourse.bass as bass  # noqa: E402
import concourse.tile as tile  # noqa: E402
from concourse import bacc, mybir  # noqa: E402
from concourse.ap import AP  # noqa: E402
from concourse.bass_utils import run_bass_kernel_spmd  # noqa: E402

H = 320
W = 320
NCORES = 8
BIG = 1e5
LENGTH = 320

F32 = mybir.dt.float32
F16 = mybir.dt.float16
Alu = mybir.AluOpType
ActFn = mybir.ActivationFunctionType

# chunk layout over 320 rows/cols of partitions
CHUNKS = [(0, 128), (128, 128), (256, 64)]

_prog_cache: dict = {}


def _denoms():
    sig = (np.float32(np.array([0.02, 0.08, 0.16], np.float32)) * np.float32(LENGTH)).astype(np.float32)
    return (np.float32(2.0) * sig * sig).astype(np.float32)


def _win(apo, col0, ni, istep, nk, kstep):
    """3D overlapping-window view of a 2D [P, F] AP: [p, i, k] -> col0 + i*istep + k*kstep."""
    return AP(apo.tensor, apo.offset + col0, [list(apo.ap[0]), [istep, ni], [kstep, nk]])


def _build(R, fp16):
    """Build + compile the per-core program. Returns the Bacc module."""
    dt = F16 if fp16 else F32
    CLAMP = 250.0 if fp16 else BIG
    PADV = 60000.0 if fp16 else 1e20
    # left pad rounded up to 16: xbar transpose dest column offset must be
    # 16-element aligned (HW constraint, unmodeled in sim)
    RP = ((R + 15) // 16) * 16
    PADH = RP + H + R
    dens = _denoms()

    nc = bacc.Bacc("TRN2", target_bir_lowering=False, debug=False, num_devices=NCORES)
    x_d = nc.dram_tensor("x", [H, W], F32, kind="ExternalInput").ap()
    iotab_d = nc.dram_tensor("iotab", [128, W], dt, kind="ExternalInput").ap()
    iotabr_d = nc.dram_tensor("iotabr", [128, W], dt, kind="ExternalInput").ap()
    o2_d = nc.dram_tensor("o2", [128, R], dt, kind="ExternalInput").ap()
    idt_d = None
    if not fp16:
        idt_d = nc.dram_tensor("idt", [128, 128], F32, kind="ExternalInput").ap()
    y_d = nc.dram_tensor("y", [3, W, H], F32, kind="ExternalOutput").ap()

    with tile.TileContext(nc) as tc:
        with (
            tc.tile_pool(name="const", bufs=1) as constp,
            tc.tile_pool(name="pa", bufs=2) as pa,
            tc.tile_pool(name="g2p", bufs=3) as g2p,
            tc.tile_pool(name="g2tp", bufs=3) as g2tp,
            tc.tile_pool(name="sbig", bufs=2) as sbig,
            tc.tile_pool(name="outp", bufs=2) as outp,
            tc.tile_pool(name="psum", bufs=4, space="PSUM") as psump,
        ):
            iotab = constp.tile([128, W], dt)
            nc.sync.dma_start(iotab[:], iotab_d)
            iotabr = constp.tile([128, W], dt)
            nc.sync.dma_start(iotabr[:], iotabr_d)
            o2 = constp.tile([128, R], dt)
            nc.sync.dma_start(o2[:], o2_d)
            idt = None
            if not fp16:
                idt = constp.tile([128, 128], F32)
                nc.sync.dma_start(idt[:], idt_d)

            # padded transposed g2 tiles, one per W-chunk
            g2t = []
            for wc, (w0, ws) in enumerate(CHUNKS):
                t = g2tp.tile([128, PADH], dt, tag="g2t")
                nc.vector.memset(t[:], PADV)
                g2t.append(t)

            # ---- phase A: row scans per H-chunk ----
            for hc, (h0, hs) in enumerate(CHUNKS):
                x_t = pa.tile([128, W], F32, tag="x")
                nc.sync.dma_start(x_t[:hs], x_d[h0 : h0 + hs, :])
                # u = 1 - x  (bit-exact: (-x) + 1)
                u_t = pa.tile([128, W], F32, tag="u")
                nc.vector.tensor_scalar(u_t[:hs], x_t[:hs], -1.0, 1.0, Alu.mult, Alu.add)
                # m = (u * 127.5) < 1.0   (== uint8 trunc zero mask)
                m_t = pa.tile([128, W], dt, tag="m")
                nc.vector.tensor_scalar(m_t[:hs], u_t[:hs], 127.5, 1.0, Alu.mult, Alu.is_lt)
                # left distances: running max of m * (w + BIGH)
                vd = pa.tile([128, W], dt, tag="vd")
                nc.vector.tensor_tensor(vd[:hs], m_t[:hs], iotab[:hs], Alu.mult)
                sL = pa.tile([128, W], dt, tag="sL")
                nc.vector.tensor_tensor_scan(
                    sL[:hs], vd[:hs], vd[:hs], 0.0, Alu.max, Alu.bypass
                )
                dL = pa.tile([128, W], dt, tag="dL")
                nc.vector.tensor_tensor(dL[:hs], iotab[:hs], sL[:hs], Alu.subtract)
                # right distances: reverse running max of m * ((W-1-w) + BIGH)
                vd2 = pa.tile([128, W], dt, tag="vd2")
                nc.vector.tensor_tensor(vd2[:hs], m_t[:hs], iotabr[:hs], Alu.mult)
                sR = pa.tile([128, W], dt, tag="sR")
                nc.vector.tensor_tensor_scan(
                    sR[:hs, ::-1], vd2[:hs, ::-1], vd2[:hs, ::-1], 0.0, Alu.max, Alu.bypass
                )
                dR = pa.tile([128, W], dt, tag="dR")
                nc.vector.tensor_tensor(dR[:hs], iotabr[:hs], sR[:hs], Alu.subtract)
                # g = min(min(dL, CLAMP), dR)
                g_t = pa.tile([128, W], dt, tag="g")
                nc.vector.scalar_tensor_tensor(
                    g_t[:hs], dL[:hs], CLAMP, dR[:hs], Alu.min, Alu.min
                )
                # g2 = g * g on ACT. Tile padded to 384 cols: xbar transpose
                # needs source free dim % 128 == 0; cols 320:384 are junk and
                # land in g2t[2] partitions 64:128, which phase B never reads.
                g2_t = g2p.tile([128, 384], dt, tag="g2")
                nc.vector.memset(g2_t[:hs, W:384], 0.0)
                nc.scalar.activation(g2_t[:hs, :W], g_t[:hs], ActFn.Square)

                # transpose blocks into g2t[wc][:, R+h0 : R+h0+hs]
                for wc, (w0, ws) in enumerate(CHUNKS):
                    if fp16:
                        nc.sync.dma_start_transpose(
                            g2t[wc][:128, RP + h0 : RP + h0 + hs],
                            g2_t[:hs, w0 : w0 + 128],
                        )
                    else:
                        pt = psump.tile([128, 128], F32, tag="pt")
                        nc.tensor.transpose(
                            pt[:ws, :hs], g2_t[:hs, w0 : w0 + ws], idt[:hs, :hs]
                        )
                        nc.scalar.copy(
                            g2t[wc][:ws, RP + h0 : RP + h0 + hs], pt[:ws, :hs]
                        )

            # ---- phase B: windowed min-plus per W-chunk ----
            for wc, (w0, ws) in enumerate(CHUNKS):
                gt = g2t[wc]
                gta = gt[:ws]
                S = sbig.tile([128, H * R], dt, tag="S")
                S3 = S[:ws].rearrange("p (i k) -> p i k", k=R)
                # S[p,i,k] = min(gt[p, R+1 + i + k], gt[p, R-1 + i - k])  (o = +-(k+1))
                in_pos = _win(gta, RP + 1, H, 1, R, 1)
                in_neg = _win(gta, RP - 1, H, 1, R, -1)
                nc.vector.tensor_tensor(S3, in_pos, in_neg, Alu.min)
                # C = S + o^2, broadcast o2 over i
                o2b = _win(o2[:ws], 0, H, 0, R, 1)
                nc.vector.tensor_tensor(S3, S3, o2b, Alu.add)
                # fold min over k
                r = R
                while r > 1:
                    h_ = r // 2
                    keep = r - h_
                    a = S3[:, :, 0:h_]
                    b = S3[:, :, keep : keep + h_]
                    nc.vector.tensor_tensor(a, a, b, Alu.min)
                    r = keep
                # d2 = min(C_folded, center o=0)
                d2 = pa.tile([128, H], dt, tag="d2")
                nc.vector.tensor_tensor(
                    d2[:ws], S3[:, :, 0], gta[:, RP : RP + H], Alu.min
                )
                # exp planes
                out_t = outp.tile([128, 3 * H], F32, tag="out")
                o3 = out_t[:ws].rearrange("p (s i) -> p s i", s=3)
                for s in range(3):
                    nc.scalar.activation(
                        o3[:, s, :], d2[:ws], ActFn.Exp, scale=float(-1.0 / dens[s])
                    )
                for s in range(3):
                    nc.sync.dma_start(y_d[s, w0 : w0 + ws, :], o3[:, s, :])

    nc.compile()
    return nc


def _host_prep(imgs):
    """Exact host-side analysis: zero mask, row distances, and the offset
    radius R needed for an exact phase B. Returns (R, fp16_ok)."""
    u = (np.float32(1.0) - imgs) * np.float32(127.5)
    m = u < np.float32(1.0)  # == (uint8 trunc == 0)
    wi = np.arange(W, dtype=np.float32)
    last = np.maximum.accumulate(np.where(m, wi, np.float32(-BIG)), axis=2)
    nxt = np.minimum.accumulate(
        np.where(m, wi, np.float32(2 * BIG))[:, :, ::-1], axis=2
    )[:, :, ::-1]
    g = np.minimum(np.minimum(wi - last, nxt - wi), np.float32(BIG)).astype(np.float32)
    g2 = g * g  # row squared distances, 1e10 where row has no seed
    seeded = m.any(axis=(1, 2))
    if not seeded.any():
        return 2, True
    D = g2.copy()
    o = 0
    while True:
        Mx = float(D[seeded].max())
        if o * o >= Mx or o >= H - 1:
            break
        o += 1
        c = np.float32(o * o)
        D[:, o:, :] = np.minimum(D[:, o:, :], g2[:, :-o, :] + c)
        D[:, :-o, :] = np.minimum(D[:, :-o, :], g2[:, o:, :] + c)
    maxd2 = float(D[seeded].max())
    R = max(2, int(math.ceil(math.sqrt(maxd2))))
    R = min(R, H - 1)
    fp16_ok = maxd2 <= 2047.0
    return R, fp16_ok


def _consts(R, fp16):
    dt = np.float16 if fp16 else np.float32
    bigh = 1024.0 if fp16 else BIG
    iotab = (np.arange(W) + bigh)[None, :].repeat(128, 0).astype(dt)
    iotabr = ((W - 1 - np.arange(W)) + bigh)[None, :].repeat(128, 0).astype(dt)
    o2 = ((np.arange(R) + 1.0) ** 2)[None, :].repeat(128, 0).astype(dt)
    out = {"iotab": iotab, "iotabr": iotabr, "o2": o2}
    if not fp16:
        out["idt"] = np.eye(128, dtype=np.float32)
    return out


def get_program(R, fp16):
    key = (R, fp16)
    if key not in _prog_cache:
        _prog_cache[key] = _build(R, fp16)
    return _prog_cache[key]


def kernel(inputs):
    inputs = np.asarray(inputs, dtype=np.float32)
    Bn = inputs.shape[0]
    # fold channel into batch: imgs[2b+c] = inputs[b, :, :, c]
    imgs = np.moveaxis(inputs, -1, 1).reshape(Bn * 2, H, W)
    assert imgs.shape[0] == NCORES, f"expected {NCORES} folded images, got {imgs.shape[0]}"

    R, fp16 = _host_prep(imgs)
    nc = get_program(R, fp16)
    cst = _consts(R, fp16)
    in_maps = [
        {"x": np.ascontiguousarray(imgs[i]), **cst} for i in range(NCORES)
    ]
    res = run_bass_kernel_spmd(nc, in_maps, list(range(NCORES)))
    out = np.empty((Bn, H, W, 6), np.float32)
    for core in range(NCORES):
        planes = res.results[core]["y"]  # [3, W, H]
        b, c = divmod(core, 2)
        for s in range(3):
            out[b, :, :, c * 3 + s] = planes[s].T
    return out


# revision 6
# speedup vs baseline: 1.0035x; 1.0035x over previous
"""Trainium2 Bass kernel for nn_Distance (exact EDT + Gaussian click maps).

Computes, for inputs [4, 320, 320, 2] f32 in [0,1):
  restored = uint8((1-x)*127.5); zero-mask = (restored == 0)
  d2 = exact squared Euclidean distance transform of the zero-mask
       (per image, channel folded into batch -> 8 independent images)
  out[..., c*3+s] = exp(-d2_c / (2*sigma_s^2)), sigmas = [0.02,0.08,0.16]*320

Sharding: pure data parallel, one folded image (b, c) per NeuronCore (8 cores).

Device algorithm (exact, decomposed transposed relative to the reference --
provably identical results):
  phase A: per-row 1D distances along W via two prefix max scans (DVE),
           g2row = g^2 (ACT square)
  transpose g2row -> [W(p), H(f)] via DMA xbar transpose (fp16) / PE (fp32)
  phase B: d2[j, i] = min_{|o|<=R} g2row_T[j, i+o] + o^2 as a bulk
           windowed tensor-tensor min + add + log-fold min tree (DVE, fp16 2x)
  exp:     3 ACT Exp activations per chunk, scale = -1/denom_s
R is derived on the host from the actual input (exact convergence bound), so
device results are exact. fp16 path is used when max finite d2 <= 2047 (all
winning candidates are integers <= 2047, exactly representable in fp16;
clamped/padded losers stay > any winner -- see analysis in comments).
"""

import math
import os
import sys

import numpy as np

for _p in ("/opt/trn_rl_repo", "/root/.axon_site/_ro/trn_rl_repo"):
    if os.path.isdir(_p) and _p not in sys.path:
        sys.path.insert(0, _p)

import concourse.bass as bass  # noqa: E402
import concourse.tile as tile  # noqa: E402
from concourse import bacc, mybir  # noqa: E402
from concourse.ap import AP  # noqa: E402
from concourse.bass_utils import run_bass_kernel_spmd  # noqa: E402

H = 320
W = 320
NCORES = 8
BIG = 1e5
LENGTH = 320

F32 = mybir.dt.float32
F16 = mybir.dt.float16
Alu = mybir.AluOpType
ActFn = mybir.ActivationFunctionType

# chunk layout over 320 rows/cols of partitions
CHUNKS = [(0, 128), (128, 128), (256, 64)]

_prog_cache: dict = {}


def _denoms():
    sig = (np.float32(np.array([0.02, 0.08, 0.16], np.float32)) * np.float32(LENGTH)).astype(np.float32)
    return (np.float32(2.0) * sig * sig).astype(np.float32)


def _win(apo, col0, ni, istep, nk, kstep):
    """3D overlapping-window view of a 2D [P, F] AP: [p, i, k] -> col0 + i*istep + k*kstep."""
    return AP(apo.tensor, apo.offset + col0, [list(apo.ap[0]), [istep, ni], [kstep, nk]])


def _build(R, fp16):
    """Build + compile the per-core program. Returns the Bacc module."""
    dt = F16 if fp16 else F32
    CLAMP = 250.0 if fp16 else BIG
    PADV = 60000.0 if fp16 else 1e20
    # left pad rounded up to 16: xbar transpose dest column offset must be
    # 16-element aligned (HW constraint, unmodeled in sim)
    RP = ((R + 15) // 16) * 16
    PADH = RP + H + R
    dens = _denoms()

    nc = bacc.Bacc("TRN2", target_bir_lowering=False, debug=False, num_devices=NCORES)
    x_d = nc.dram_tensor("x", [H, W], F32, kind="ExternalInput").ap()
    iotab_d = nc.dram_tensor("iotab", [128, W], dt, kind="ExternalInput").ap()
    iotabr_d = nc.dram_tensor("iotabr", [128, W], dt, kind="ExternalInput").ap()
    o2_d = nc.dram_tensor("o2", [128, R + 1], dt, kind="ExternalInput").ap()
    idt_d = None
    if not fp16:
        idt_d = nc.dram_tensor("idt", [128, 128], F32, kind="ExternalInput").ap()
    y_d = nc.dram_tensor("y", [3, W, H], F32, kind="ExternalOutput").ap()

    with tile.TileContext(nc) as tc:
        with (
            tc.tile_pool(name="const", bufs=1) as constp,
            tc.tile_pool(name="pa", bufs=2) as pa,
            tc.tile_pool(name="g2p", bufs=3) as g2p,
            tc.tile_pool(name="g2tp", bufs=3) as g2tp,
            tc.tile_pool(name="sbig", bufs=2) as sbig,
            tc.tile_pool(name="outp", bufs=2) as outp,
            tc.tile_pool(name="psum", bufs=4, space="PSUM") as psump,
        ):
            iotab = constp.tile([128, W], dt)
            nc.sync.dma_start(iotab[:], iotab_d)
            iotabr = constp.tile([128, W], dt)
            nc.sync.dma_start(iotabr[:], iotabr_d)
            o2 = constp.tile([128, R + 1], dt)
            nc.sync.dma_start(o2[:], o2_d)
            idt = None
            if not fp16:
                idt = constp.tile([128, 128], F32)
                nc.sync.dma_start(idt[:], idt_d)

            # padded transposed g2 tiles, one per W-chunk
            g2t = []
            for wc, (w0, ws) in enumerate(CHUNKS):
                t = g2tp.tile([128, PADH], dt, tag="g2t")
                nc.gpsimd.memset(t[:], PADV)
                g2t.append(t)

            # ---- phase A: row scans per H-chunk ----
            for hc, (h0, hs) in enumerate(CHUNKS):
                x_t = pa.tile([128, W], F32, tag="x")
                nc.sync.dma_start(x_t[:hs], x_d[h0 : h0 + hs, :])
                # u = 1 - x  (bit-exact: (-x) + 1)
                u_t = pa.tile([128, W], F32, tag="u")
                nc.gpsimd.tensor_scalar(u_t[:hs], x_t[:hs], -1.0, 1.0, Alu.mult, Alu.add)
                # m = (u * 127.5) < 1.0   (== uint8 trunc zero mask)
                m_t = pa.tile([128, W], dt, tag="m")
                nc.gpsimd.tensor_scalar(m_t[:hs], u_t[:hs], 127.5, 1.0, Alu.mult, Alu.is_lt)
                # left distances: running max of m * (w + BIGH)
                vd = pa.tile([128, W], dt, tag="vd")
                nc.vector.tensor_tensor(vd[:hs], m_t[:hs], iotab[:hs], Alu.mult)
                sL = pa.tile([128, W], dt, tag="sL")
                nc.vector.tensor_tensor_scan(
                    sL[:hs], vd[:hs], vd[:hs], 0.0, Alu.max, Alu.bypass
                )
                dL = pa.tile([128, W], dt, tag="dL")
                nc.vector.tensor_tensor(dL[:hs], iotab[:hs], sL[:hs], Alu.subtract)
                # right distances: reverse running max of m * ((W-1-w) + BIGH)
                vd2 = pa.tile([128, W], dt, tag="vd2")
                nc.vector.tensor_tensor(vd2[:hs], m_t[:hs], iotabr[:hs], Alu.mult)
                sR = pa.tile([128, W], dt, tag="sR")
                nc.vector.tensor_tensor_scan(
                    sR[:hs, ::-1], vd2[:hs, ::-1], vd2[:hs, ::-1], 0.0, Alu.max, Alu.bypass
                )
                dR = pa.tile([128, W], dt, tag="dR")
                nc.vector.tensor_tensor(dR[:hs], iotabr[:hs], sR[:hs], Alu.subtract)
                # g = min(min(dL, CLAMP), dR)
                g_t = pa.tile([128, W], dt, tag="g")
                nc.vector.scalar_tensor_tensor(
                    g_t[:hs], dL[:hs], CLAMP, dR[:hs], Alu.min, Alu.min
                )
                # g2 = g * g on ACT. Tile padded to 384 cols: xbar transpose
                # needs source free dim % 128 == 0; cols 320:384 are junk and
                # land in g2t[2] partitions 64:128, which phase B never reads.
                g2_t = g2p.tile([128, 384], dt, tag="g2")
                nc.gpsimd.memset(g2_t[:hs, W:384], 0.0)
                nc.scalar.activation(g2_t[:hs, :W], g_t[:hs], ActFn.Square)

                # transpose blocks into g2t[wc][:, R+h0 : R+h0+hs]
                for wc, (w0, ws) in enumerate(CHUNKS):
                    if fp16:
                        nc.sync.dma_start_transpose(
                            g2t[wc][:128, RP + h0 : RP + h0 + hs],
                            g2_t[:hs, w0 : w0 + 128],
                        )
                    else:
                        pt = psump.tile([128, 128], F32, tag="pt")
                        nc.tensor.transpose(
                            pt[:ws, :hs], g2_t[:hs, w0 : w0 + ws], idt[:hs, :hs]
                        )
                        nc.scalar.copy(
                            g2t[wc][:ws, RP + h0 : RP + h0 + hs], pt[:ws, :hs]
                        )

            # ---- phase B: windowed min-plus per W-chunk ----
            # k = 0..R (o = +-k, k=0 gives the center candidate); the k range
            # is split between DVE (fast) and GPSIMD (slower, but otherwise
            # idle), each builds min(g2t[i+k], g2t[i-k]) + k^2 and folds a
            # min tree; a final DVE min combines the two partial results.
            KALL = R + 1

            def fold_min(eng, S3, r):
                while r > 1:
                    h_ = r // 2
                    keep = r - h_
                    eng.tensor_tensor(
                        S3[:, :, 0:h_], S3[:, :, 0:h_], S3[:, :, keep : keep + h_], Alu.min
                    )
                    r = keep

            for wc, (w0, ws) in enumerate(CHUNKS):
                gt = g2t[wc]
                gta = gt[:ws]
                Sd = sbig.tile([128, H * KALL], dt, tag="Sd")
                Sd3 = Sd[:ws].rearrange("p (i k) -> p i k", k=KALL)
                nc.vector.tensor_tensor(
                    Sd3, _win(gta, RP, H, 1, KALL, 1), _win(gta, RP, H, 1, KALL, -1), Alu.min
                )
                nc.vector.tensor_tensor(Sd3, Sd3, _win(o2[:ws], 0, H, 0, KALL, 1), Alu.add)
                fold_min(nc.vector, Sd3, KALL)
                # exp planes read the folded result directly (strided)
                out_t = outp.tile([128, 3 * H], F32, tag="out")
                o3 = out_t[:ws].rearrange("p (s i) -> p s i", s=3)
                for s in range(3):
                    nc.scalar.activation(
                        o3[:, s, :], Sd3[:, :, 0], ActFn.Exp, scale=float(-1.0 / dens[s])
                    )
                for s in range(3):
                    nc.sync.dma_start(y_d[s, w0 : w0 + ws, :], o3[:, s, :])

    nc.compile()
    return nc


def _host_prep(imgs):
    """Exact host-side analysis: zero mask, row distances, and the offset
    radius R needed for an exact phase B. Returns (R, fp16_ok)."""
    u = (np.float32(1.0) - imgs) * np.float32(127.5)
    m = u < np.float32(1.0)  # == (uint8 trunc == 0)
    wi = np.arange(W, dtype=np.float32)
    last = np.maximum.accumulate(np.where(m, wi, np.float32(-BIG)), axis=2)
    nxt = np.minimum.accumulate(
        np.where(m, wi, np.float32(2 * BIG))[:, :, ::-1], axis=2
    )[:, :, ::-1]
    g = np.minimum(np.minimum(wi - last, nxt - wi), np.float32(BIG)).astype(np.float32)
    g2 = g * g  # row squared distances, 1e10 where row has no seed
    seeded = m.any(axis=(1, 2))
    if not seeded.any():
        return 2, True
    D = g2.copy()
    o = 0
    while True:
        Mx = float(D[seeded].max())
        if o * o >= Mx or o >= H - 1:
            break
        o += 1
        c = np.float32(o * o)
        D[:, o:, :] = np.minimum(D[:, o:, :], g2[:, :-o, :] + c)
        D[:, :-o, :] = np.minimum(D[:, :-o, :], g2[:, o:, :] + c)
    maxd2 = float(D[seeded].max())
    R = max(2, int(math.ceil(math.sqrt(maxd2))))
    R = min(R, H - 1)
    fp16_ok = maxd2 <= 2047.0
    return R, fp16_ok


def _consts(R, fp16):
    dt = np.float16 if fp16 else np.float32
    bigh = 1024.0 if fp16 else BIG
    iotab = (np.arange(W) + bigh)[None, :].repeat(128, 0).astype(dt)
    iotabr = ((W - 1 - np.arange(W)) + bigh)[None, :].repeat(128, 0).astype(dt)
    o2 = (np.arange(R + 1.0) ** 2)[None, :].repeat(128, 0).astype(dt)
    out = {"iotab": iotab, "iotabr": iotabr, "o2": o2}
    if not fp16:
        out["idt"] = np.eye(128, dtype=np.float32)
    return out


def get_program(R, fp16):
    key = (R, fp16)
    if key not in _prog_cache:
        _prog_cache[key] = _build(R, fp16)
    return _prog_cache[key]


def kernel(inputs):
    inputs = np.asarray(inputs, dtype=np.float32)
    Bn = inputs.shape[0]
    # fold channel into batch: imgs[2b+c] = inputs[b, :, :, c]
    imgs = np.moveaxis(inputs, -1, 1).reshape(Bn * 2, H, W)
    assert imgs.shape[0] == NCORES, f"expected {NCORES} folded images, got {imgs.shape[0]}"

    R, fp16 = _host_prep(imgs)
    nc = get_program(R, fp16)
    cst = _consts(R, fp16)
    in_maps = [
        {"x": np.ascontiguousarray(imgs[i]), **cst} for i in range(NCORES)
    ]
    res = run_bass_kernel_spmd(nc, in_maps, list(range(NCORES)))
    out = np.empty((Bn, H, W, 6), np.float32)
    for core in range(NCORES):
        planes = res.results[core]["y"]  # [3, W, H]
        b, c = divmod(core, 2)
        for s in range(3):
            out[b, :, :, c * 3 + s] = planes[s].T
    return out
